# revision 20
# baseline (speedup 1.0000x reference)
"""Causal GQA self-attention (b=4, s=2048, dim=2048, 16 q-heads / 4 kv-heads,
hd=128, RoPE) on 8 TRN2 NeuronCores.

Sharding: tensor-parallel x2 on heads x data-parallel x4 on batch.
Core c <-> (batch c//2, head-half c%2). Each core:
  - projects Q (8 heads) / K,V (2 kv heads) for all 2048 tokens of its batch
    directly in [d, t] layout (host feeds x^T and W^T pre-cast to bf16),
  - applies RoPE via a partition-permutation matmul + DVE combines,
  - computes attention in S^T = K^T.T@Q layout (k on partitions, q on free)
    so softmax needs no transposes: exp on ScalarE, row-sums via a
    ones-column matmul accumulated in PSUM, PV accumulated in PSUM,
  - exchanges attention-output head-halves with its pair via AllToAll,
  - runs the output projection for its token half (1024 tokens), full dim.
Host reassembles the 8 (1024, 2048) f32 slabs.
"""

import os

import numpy as np
import ml_dtypes

import concourse.bass as bass
import concourse.bacc as bacc
import concourse.mybir as mybir
import concourse.tile as tile
from concourse.bass_utils import run_bass_kernel_spmd

BF16 = ml_dtypes.bfloat16
F32 = mybir.dt.float32
BF = mybir.dt.bfloat16

# Problem constants
B, S, DIM = 4, 2048, 2048
NH, NKV, HD = 16, 4, 128
ROPE_BASE = 10000.0
N_CORES = 8

# Per-core layout
NH_LOC = NH // 2          # 8 q heads per core
NKV_LOC = NKV // 2        # 2 kv heads per core
P = 128
NDT = DIM // P            # 16 contraction tiles
TQ = 512                  # q-chunk (free dim of attention matmuls)
NQT = S // TQ             # 4 q-chunks
NTT = S // P              # 16 token tiles of 128
THALF = S // 2            # 1024 tokens per core's output slab
SCALE = 1.0 / float(np.sqrt(HD))

_PROGRAM_CACHE = {}


def _build_program():
    if "nc" in _PROGRAM_CACHE:
        return _PROGRAM_CACHE["nc"]

    nc = bacc.Bacc("TRN2", target_bir_lowering=False, debug=False,
                   num_devices=N_CORES)

    xT_d = nc.dram_tensor("xT", [DIM, S], BF, kind="ExternalInput")
    wqT_d = nc.dram_tensor("wqT", [DIM, NH_LOC * HD], BF, kind="ExternalInput")
    wkT_d = nc.dram_tensor("wkT", [DIM, NKV_LOC * HD], BF, kind="ExternalInput")
    wvT_d = nc.dram_tensor("wvT", [DIM, NKV_LOC * HD], BF, kind="ExternalInput")
    woT_d = nc.dram_tensor("woT", [DIM, DIM], BF, kind="ExternalInput")
    cos_d = nc.dram_tensor("cos", [P, S], BF, kind="ExternalInput")
    ssin_d = nc.dram_tensor("ssin", [P, S], BF, kind="ExternalInput")
    mask_d = nc.dram_tensor("mask", [P, 1280], BF, kind="ExternalInput")
    perm_d = nc.dram_tensor("perm", [P, P], BF, kind="ExternalInput")
    ones_d = nc.dram_tensor("ones", [P, 1], BF, kind="ExternalInput")
    tokoff_d = nc.dram_tensor("tokoff", [1, 1], mybir.dt.uint32,
                              kind="ExternalInput")
    out_d = nc.dram_tensor("out", [THALF, DIM], mybir.dt.float32,
                           kind="ExternalOutput")

    with tile.TileContext(nc) as tc:
        # ---- constants (live for the whole kernel) ----
        with tc.tile_pool(name="const", bufs=1) as constp:
            cos_sb = constp.tile([P, S], BF, name="cos_sb")
            ssin_sb = constp.tile([P, S], BF, name="ssin_sb")
            mask_sb = constp.tile([P, 1280], BF, name="mask_sb")
            perm_sb = constp.tile([P, P], BF, name="perm_sb")
            ones_sb = constp.tile([P, 1], BF, name="ones_sb")
            tokoff_sb = constp.tile([1, 1], mybir.dt.uint32, name="tokoff_sb")
            nc.sync.dma_start(out=cos_sb[:, :], in_=cos_d[:, :])
            nc.sync.dma_start(out=ssin_sb[:, :], in_=ssin_d[:, :])
            nc.sync.dma_start(out=mask_sb[:, :], in_=mask_d[:, :])
            nc.sync.dma_start(out=perm_sb[:, :], in_=perm_d[:, :])
            nc.sync.dma_start(out=ones_sb[:, :], in_=ones_d[:, :])
            nc.sync.dma_start(out=tokoff_sb[:, :], in_=tokoff_d[:, :])

            # ---- persistent activation buffers ----
            with tc.tile_pool(name="acts", bufs=1) as actp:
                q_sb = [actp.tile([P, S], BF, name=f"q{h}") for h in range(NH_LOC)]
                k_sb = [actp.tile([P, S], BF, name=f"k{g}") for g in range(NKV_LOC)]
                v_sb = [actp.tile([P, NKV_LOC * HD], BF, name=f"v{t}")
                        for t in range(NTT)]

                # =========== projections + RoPE ===========
                with tc.tile_pool(name="proj_in", bufs=1) as pin, \
                     tc.tile_pool(name="proj_ps", bufs=4, space="PSUM") as pps, \
                     tc.tile_pool(name="proj_wk", bufs=3) as pwk:
                    xt = [pin.tile([P, S], BF, name=f"xt{i}") for i in range(NDT)]
                    wq = [pin.tile([P, NH_LOC * HD], BF, name=f"wq{i}")
                          for i in range(NDT)]
                    wk = [pin.tile([P, NKV_LOC * HD], BF, name=f"wk{i}")
                          for i in range(NDT)]
                    wv = [pin.tile([P, NKV_LOC * HD], BF, name=f"wv{i}")
                          for i in range(NDT)]
                    for i in range(NDT):
                        nc.sync.dma_start(out=xt[i][:, :],
                                          in_=xT_d[i * P:(i + 1) * P, :])
                        nc.sync.dma_start(out=wq[i][:, :],
                                          in_=wqT_d[i * P:(i + 1) * P, :])
                        nc.sync.dma_start(out=wk[i][:, :],
                                          in_=wkT_d[i * P:(i + 1) * P, :])
                        nc.sync.dma_start(out=wv[i][:, :],
                                          in_=wvT_d[i * P:(i + 1) * P, :])

                    def rope_head(w_tiles, col0, dst):
                        # dst[:, c*TQ:+TQ] = rope(sum_D w.T @ x) for one head
                        for c in range(NQT):
                            ps = pps.tile([P, TQ], F32, name="projps", tag="projps",
                                          bufs=4)
                            for d in range(NDT):
                                nc.tensor.matmul(
                                    ps[:, :],
                                    lhsT=w_tiles[d][:, col0:col0 + HD],
                                    rhs=xt[d][:, c * TQ:(c + 1) * TQ],
                                    start=(d == 0), stop=(d == NDT - 1))
                            raw = pwk.tile([P, TQ], BF, name="rope_raw",
                                           tag="rope_raw", bufs=3)
                            nc.scalar.copy(raw[:, :], ps[:, :])
                            pperm = pps.tile([P, TQ], F32, name="permps",
                                             tag="permps", bufs=2)
                            nc.tensor.matmul(pperm[:, :], lhsT=perm_sb[:, :],
                                             rhs=raw[:, :], start=True, stop=True)
                            t1 = pwk.tile([P, TQ], BF, name="rope_t1",
                                          tag="rope_t1", bufs=3)
                            nc.vector.tensor_mul(
                                t1[:, :], raw[:, :],
                                cos_sb[:, c * TQ:(c + 1) * TQ])
                            t2 = pwk.tile([P, TQ], BF, name="rope_t2",
                                          tag="rope_t2", bufs=3)
                            nc.vector.tensor_mul(
                                t2[:, :], pperm[:, :],
                                ssin_sb[:, c * TQ:(c + 1) * TQ])
                            nc.vector.tensor_add(
                                dst[:, c * TQ:(c + 1) * TQ], t1[:, :], t2[:, :])

                    for h in range(NH_LOC):
                        rope_head(wq, h * HD, q_sb[h])
                    for g in range(NKV_LOC):
                        rope_head(wk, g * HD, k_sb[g])

                    # V token-major: out[t_tile, dv] = x^T_tile.T @ wvT
                    for t in range(NTT):
                        ps = pps.tile([P, NKV_LOC * HD], F32, name="vps",
                                      tag="projps", bufs=4)
                        for d in range(NDT):
                            nc.tensor.matmul(
                                ps[:, :],
                                lhsT=xt[d][:, t * P:(t + 1) * P],
                                rhs=wv[d][:, :],
                                start=(d == 0), stop=(d == NDT - 1))
                        nc.scalar.copy(v_sb[t][:, :], ps[:, :])

                # =========== output-proj weights prefetch ===========
                N_CHUNK = 4
                HPC = NH_LOC // N_CHUNK          # heads per AG chunk (2)
                with tc.tile_pool(name="wo_pool", bufs=1) as wop, \
                     tc.tile_pool(name="a2a_dram", bufs=1, space="DRAM") as dramp:
                    wo = [wop.tile([P, DIM], BF, name=f"wo{i}") for i in range(NDT)]
                    for i in range(NDT):
                        nc.sync.dma_start(out=wo[i][:, :],
                                          in_=woT_d[i * P:(i + 1) * P, :])

                    ag_in = [dramp.tile([HPC * HD, S], BF, name=f"ag_in{c}")
                             for c in range(N_CHUNK)]
                    ag_out = [dramp.tile([2 * HPC * HD, S], BF, name=f"ag_out{c}")
                              for c in range(N_CHUNK)]

                    treg = nc.sync.alloc_register("tokoff_reg")
                    nc.sync.reg_load(treg, tokoff_sb[0:1, 0:1])
                    tok0 = nc.sync.snap(treg, donate=True, min_val=0,
                                        max_val=THALF)

                    # =========== attention (h-outer) + chunked AllGather ======
                    with tc.tile_pool(name="att_ps", bufs=2, space="PSUM") as aps, \
                         tc.tile_pool(name="att_wk", bufs=3) as awk, \
                         tc.tile_pool(name="at_pool", bufs=1) as atp, \
                         tc.tile_pool(name="op_ps", bufs=2, space="PSUM") as ops, \
                         tc.tile_pool(name="op_wk", bufs=4) as owk:
                        for h in range(NH_LOC):
                            g = h // (NH_LOC // NKV_LOC)
                            for qt in range(NQT):
                                nk = (qt + 1) * (TQ // P)
                                sums = aps.tile([1, TQ], F32, name="sums",
                                                tag="sums", bufs=1)
                                opv = aps.tile([P, TQ], F32, name="opv",
                                               tag="opv", bufs=2)
                                for kp in range(nk // 2):
                                    kt0 = 2 * kp
                                    sTw = aps.tile([P, 2 * TQ], F32, name="sTw",
                                                   tag="sTw", bufs=2)
                                    for j in range(2):
                                        kt = kt0 + j
                                        nc.tensor.matmul(
                                            sTw[:, j * TQ:(j + 1) * TQ],
                                            lhsT=k_sb[g][:, kt * P:(kt + 1) * P],
                                            rhs=q_sb[h][:, qt * TQ:(qt + 1) * TQ],
                                            start=True, stop=True)
                                    pTw = awk.tile([P, 2 * TQ], BF, name="pTw",
                                                   tag="pTw", bufs=3)
                                    nc.scalar.activation(
                                        pTw[:, :], sTw[:, :],
                                        mybir.ActivationFunctionType.Exp,
                                        scale=SCALE)
                                    for j in range(2):
                                        kt = kt0 + j
                                        if kt >= nk - (TQ // P):
                                            o0 = 384 - P * (kt - (nk - (TQ // P)))
                                            nc.vector.tensor_mul(
                                                pTw[:, j * TQ:(j + 1) * TQ],
                                                pTw[:, j * TQ:(j + 1) * TQ],
                                                mask_sb[:, o0:o0 + TQ])
                                    for j in range(2):
                                        kt = kt0 + j
                                        nc.tensor.matmul(
                                            sums[:, :], lhsT=ones_sb[:, :],
                                            rhs=pTw[:, j * TQ:(j + 1) * TQ],
                                            start=(kt == 0), stop=(kt == nk - 1))
                                        nc.tensor.matmul(
                                            opv[:, :],
                                            lhsT=v_sb[kt][:, g * HD:(g + 1) * HD],
                                            rhs=pTw[:, j * TQ:(j + 1) * TQ],
                                            start=(kt == 0), stop=(kt == nk - 1))
                                rec = awk.tile([1, TQ], F32, name="rec",
                                               tag="rec", bufs=2)
                                nc.vector.reciprocal_approx_fast(
                                    rec[:, :], sums[:, :])
                                recb = awk.tile([P, TQ], F32, name="recb",
                                                tag="recb", bufs=2)
                                nc.gpsimd.partition_broadcast(
                                    recb[:, :], rec[0:1, :])
                                att = awk.tile([P, TQ], BF, name="att",
                                               tag="att", bufs=4)
                                nc.vector.tensor_mul(
                                    att[:, :], opv[:, :], recb[:, :])
                                c = h // HPC
                                nc.sync.dma_start(
                                    out=ag_in[c][(h % HPC) * HD:
                                                 (h % HPC + 1) * HD,
                                                 qt * TQ:(qt + 1) * TQ],
                                    in_=att[:, :])

                            # chunk complete -> exchange with the pair
                            if h % HPC == HPC - 1:
                                c = h // HPC
                                nc.gpsimd.collective_compute(
                                    "AllGather",
                                    mybir.AluOpType.bypass,
                                    replica_groups=[[2 * i, 2 * i + 1]
                                                    for i in range(4)],
                                    ins=[ag_in[c].opt()],
                                    outs=[ag_out[c].opt()],
                                )

                        # ===== output projection (my token half) =====
                        # F tile f: rank r=f//8, chunk c=(f%8)//HPC,
                        # row (f%8)%HPC within the chunk's rank block.
                        at = [atp.tile([P, THALF], BF, name=f"at{f}")
                              for f in range(NDT)]
                        for f in range(NDT):
                            r, fl = f // (NDT // 2), f % (NDT // 2)
                            c, row = fl // HPC, fl % HPC
                            nc.sync.dma_start(
                                out=at[f][:, :],
                                in_=ag_out[c][r * HPC * HD + row * P:
                                              r * HPC * HD + (row + 1) * P,
                                              bass.ds(tok0, THALF)])
                        for tt in range(8):
                            t0 = tt * P
                            for eh in range(2):
                                ps2 = [ops.tile([P, TQ], F32, name="opps",
                                                tag="opps", bufs=1)
                                       for _ in range(2)]
                                for f in range(NDT):
                                    for j in range(2):
                                        e4 = eh * 2 + j
                                        nc.tensor.matmul(
                                            ps2[j][:, :],
                                            lhsT=at[f][:, t0:t0 + P],
                                            rhs=wo[f][:, e4 * TQ:(e4 + 1) * TQ],
                                            start=(f == 0), stop=(f == NDT - 1))
                                for j in range(2):
                                    e4 = eh * 2 + j
                                    osb = owk.tile([P, TQ], mybir.dt.float32,
                                                   name="osb", tag="osb", bufs=4)
                                    nc.scalar.copy(osb[:, :], ps2[j][:, :])
                                    nc.sync.dma_start(
                                        out=out_d[t0:t0 + P,
                                                  e4 * TQ:(e4 + 1) * TQ],
                                        in_=osb[:, :])

    nc.compile()
    _PROGRAM_CACHE["nc"] = nc
    return nc


def _host_tables():
    inv_freq = 1.0 / (ROPE_BASE ** (np.arange(0, HD, 2, dtype=np.float64) / HD))
    pos = np.arange(S, dtype=np.float64)
    ang = pos[None, :] * inv_freq[:, None]          # [64, S]
    cos = np.concatenate([np.cos(ang), np.cos(ang)], axis=0)   # [128, S]
    sin = np.sin(ang)
    ssin = np.concatenate([-sin, sin], axis=0)                  # [128, S]

    kk = np.arange(P)[:, None]
    cc = np.arange(1280)[None, :]
    mask = (kk <= cc - 384).astype(np.float32)                  # [128, 1280]

    permm = np.roll(np.eye(P, dtype=np.float32), 64, axis=0)    # q[(d+64)%128]
    ones = np.ones((P, 1), np.float32)
    return (cos.astype(BF16), ssin.astype(BF16), mask.astype(BF16),
            permm.astype(BF16), ones.astype(BF16))


def kernel(x, Wq, Wkv, Wo):
    x = np.asarray(x, np.float32)
    Wq = np.asarray(Wq, np.float32)
    Wkv = np.asarray(Wkv, np.float32)
    Wo = np.asarray(Wo, np.float32)

    nc = _build_program()
    cos, ssin, mask, permm, ones = _host_tables()
    wqT = np.ascontiguousarray(Wq.T).astype(BF16)       # [DIM, 2048]
    wkvT = np.ascontiguousarray(Wkv.T).astype(BF16)     # [DIM, 1024]
    woT = np.ascontiguousarray(Wo.T).astype(BF16)       # [DIM, DIM]

    in_maps = []
    for c in range(N_CORES):
        b, hh = c // 2, c % 2
        xT = np.ascontiguousarray(x[b].T).astype(BF16)  # [DIM, S]
        in_maps.append({
            "xT": xT,
            "wqT": np.ascontiguousarray(
                wqT[:, hh * NH_LOC * HD:(hh + 1) * NH_LOC * HD]),
            "wkT": np.ascontiguousarray(
                wkvT[:, hh * NKV_LOC * HD:(hh + 1) * NKV_LOC * HD]),
            "wvT": np.ascontiguousarray(
                wkvT[:, NKV * HD + hh * NKV_LOC * HD:
                     NKV * HD + (hh + 1) * NKV_LOC * HD]),
            "woT": woT,
            "cos": cos, "ssin": ssin, "mask": mask,
            "perm": permm, "ones": ones,
            "tokoff": np.array([[hh * THALF]], np.uint32),
        })

    trace_kwargs = {}
    if os.environ.get("KERNEL_TRACE") == "1":
        trace_kwargs = dict(trace=True,
                            trace_cores=list(range(N_CORES)),
                            stitch_traces=True)
    elif os.environ.get("KERNEL_TRACE") == "0cores":
        trace_kwargs = dict(trace=True)
    res = run_bass_kernel_spmd(nc, in_maps, core_ids=list(range(N_CORES)),
                               **trace_kwargs)
    _PROGRAM_CACHE["last_results"] = res

    out = np.empty((B, S, DIM), np.float32)
    for c in range(N_CORES):
        b, hh = c // 2, c % 2
        out[b, hh * THALF:(hh + 1) * THALF, :] = res.results[c]["out"]
    return out


# revision 22
# speedup vs baseline: 1.1024x; 1.1024x over previous
"""Causal GQA self-attention (b=4, s=2048, dim=2048, 16 q-heads / 4 kv-heads,
hd=128, RoPE) on 8 TRN2 NeuronCores.

Sharding: tensor-parallel x2 on heads x data-parallel x4 on batch.
Core c <-> (batch c//2, head-half c%2). Each core:
  - projects Q (8 heads) / K,V (2 kv heads) for all 2048 tokens of its batch
    directly in [d, t] layout (host feeds x^T and W^T pre-cast to bf16),
  - applies RoPE via a partition-permutation matmul + DVE combines,
  - computes attention in S^T = K^T.T@Q layout (k on partitions, q on free)
    so softmax needs no transposes: exp on ScalarE, row-sums via a
    ones-column matmul accumulated in PSUM, PV accumulated in PSUM.
    Q-projection of head h+1 is interleaved with attention of head h so
    projection matmuls fill PE stall slots in the attention pipeline,
  - exchanges attention-output head-halves with its pair via 4 chunked
    AllGathers overlapped with attention,
  - runs the output projection for its token half (1024 tokens), full dim;
    the token offset comes from a per-core uint32 input via a dynamic DMA
    offset so the program is identical across cores (SPMD).
Host reassembles the 8 (1024, 2048) f32 slabs.
"""

import os

import numpy as np
import ml_dtypes

import concourse.bass as bass
import concourse.bacc as bacc
import concourse.mybir as mybir
import concourse.tile as tile
from concourse.bass_utils import run_bass_kernel_spmd

BF16 = ml_dtypes.bfloat16
F32 = mybir.dt.float32
BF = mybir.dt.bfloat16

# Problem constants
B, S, DIM = 4, 2048, 2048
NH, NKV, HD = 16, 4, 128
ROPE_BASE = 10000.0
N_CORES = 8

# Per-core layout
NH_LOC = NH // 2          # 8 q heads per core
NKV_LOC = NKV // 2        # 2 kv heads per core
P = 128
NDT = DIM // P            # 16 contraction tiles
TQ = 512                  # q-chunk (free dim of attention matmuls)
NQT = S // TQ             # 4 q-chunks
NTT = S // P              # 16 token tiles of 128
THALF = S // 2            # 1024 tokens per core's output slab
SCALE = 1.0 / float(np.sqrt(HD))
N_CHUNK = 4               # AllGather chunks
HPC = NH_LOC // N_CHUNK   # heads per chunk

_PROGRAM_CACHE = {}


def _build_program():
    if "nc" in _PROGRAM_CACHE:
        return _PROGRAM_CACHE["nc"]

    nc = bacc.Bacc("TRN2", target_bir_lowering=False, debug=False,
                   num_devices=N_CORES)

    xT_d = nc.dram_tensor("xT", [DIM, S], BF, kind="ExternalInput")
    wqT_d = nc.dram_tensor("wqT", [DIM, NH_LOC * HD], BF, kind="ExternalInput")
    wkT_d = nc.dram_tensor("wkT", [DIM, NKV_LOC * HD], BF, kind="ExternalInput")
    wvT_d = nc.dram_tensor("wvT", [DIM, NKV_LOC * HD], BF, kind="ExternalInput")
    woT_d = nc.dram_tensor("woT", [DIM, DIM], BF, kind="ExternalInput")
    cos_d = nc.dram_tensor("cos", [P, S], BF, kind="ExternalInput")
    ssin_d = nc.dram_tensor("ssin", [P, S], BF, kind="ExternalInput")
    mask_d = nc.dram_tensor("mask", [P, 1280], BF, kind="ExternalInput")
    perm_d = nc.dram_tensor("perm", [P, P], BF, kind="ExternalInput")
    ones_d = nc.dram_tensor("ones", [P, 1], BF, kind="ExternalInput")
    tokoff_d = nc.dram_tensor("tokoff", [1, 1], mybir.dt.uint32,
                              kind="ExternalInput")
    out_d = nc.dram_tensor("out", [THALF, DIM], mybir.dt.float32,
                           kind="ExternalOutput")

    with tile.TileContext(nc) as tc:
        with tc.tile_pool(name="const", bufs=1) as constp, \
             tc.tile_pool(name="acts", bufs=1) as actp, \
             tc.tile_pool(name="dram", bufs=1, space="DRAM") as dramp, \
             tc.tile_pool(name="main_ps", bufs=2, space="PSUM") as pps, \
             tc.tile_pool(name="wk", bufs=3) as pwk:
            # ---- constants ----
            cos_sb = constp.tile([P, S], BF, name="cos_sb")
            ssin_sb = constp.tile([P, S], BF, name="ssin_sb")
            mask_sb = constp.tile([P, 1280], BF, name="mask_sb")
            perm_sb = constp.tile([P, P], BF, name="perm_sb")
            ones_sb = constp.tile([P, 1], BF, name="ones_sb")
            tokoff_sb = constp.tile([1, 1], mybir.dt.uint32, name="tokoff_sb")
            nc.sync.dma_start(out=cos_sb[:, :], in_=cos_d[:, :])
            nc.sync.dma_start(out=ssin_sb[:, :], in_=ssin_d[:, :])
            nc.sync.dma_start(out=mask_sb[:, :], in_=mask_d[:, :])
            nc.sync.dma_start(out=perm_sb[:, :], in_=perm_d[:, :])
            nc.sync.dma_start(out=ones_sb[:, :], in_=ones_d[:, :])
            nc.sync.dma_start(out=tokoff_sb[:, :], in_=tokoff_d[:, :])

            treg = nc.sync.alloc_register("tokoff_reg")
            nc.sync.reg_load(treg, tokoff_sb[0:1, 0:1])
            tok0 = nc.sync.snap(treg, donate=True, min_val=0, max_val=THALF)

            # ---- persistent activations ----
            q_sb = [actp.tile([P, S], BF, name=f"q{h}") for h in range(NH_LOC)]
            k_sb = [actp.tile([P, S], BF, name=f"k{g}") for g in range(NKV_LOC)]
            v_sb = [actp.tile([P, NKV_LOC * HD], BF, name=f"v{t}")
                    for t in range(NTT)]

            ag_in = [dramp.tile([HPC * HD, S], BF, name=f"ag_in{c}")
                     for c in range(N_CHUNK)]
            ag_out = [dramp.tile([2 * HPC * HD, S], BF, name=f"ag_out{c}")
                      for c in range(N_CHUNK)]

            # ---- streamed inputs as stack-ordered singles ----
            frees = []
            xt, wqt, wkt, wvt = [], [], [], []
            for i in range(NDT):
                t_, f_ = tc.tile([P, S], BF, name=f"xt{i}")
                xt.append(t_); frees.append(f_)
            for i in range(NDT):
                t_, f_ = tc.tile([P, NH_LOC * HD], BF, name=f"wq{i}")
                wqt.append(t_); frees.append(f_)
            for i in range(NDT):
                t_, f_ = tc.tile([P, NKV_LOC * HD], BF, name=f"wk{i}")
                wkt.append(t_); frees.append(f_)
                t_, f_ = tc.tile([P, NKV_LOC * HD], BF, name=f"wv{i}")
                wvt.append(t_); frees.append(f_)
            for i in range(NDT):
                nc.sync.dma_start(out=xt[i][:, :], in_=xT_d[i * P:(i + 1) * P, :])
                nc.sync.dma_start(out=wkt[i][:, :], in_=wkT_d[i * P:(i + 1) * P, :])
                nc.sync.dma_start(out=wvt[i][:, :], in_=wvT_d[i * P:(i + 1) * P, :])
                nc.sync.dma_start(out=wqt[i][:, :], in_=wqT_d[i * P:(i + 1) * P, :])

            # PSUM tags: projps(2) + permps(1) + sT(2) + sums(1) + opv(2) = 8
            def rope_head(w_tiles, col0, dst):
                for c in range(NQT):
                    ps = pps.tile([P, TQ], F32, name="projps",
                                  tag="projps", bufs=2)
                    for d in range(NDT):
                        nc.tensor.matmul(
                            ps[:, :],
                            lhsT=w_tiles[d][:, col0:col0 + HD],
                            rhs=xt[d][:, c * TQ:(c + 1) * TQ],
                            start=(d == 0), stop=(d == NDT - 1))
                    raw = pwk.tile([P, TQ], BF, name="rope_raw",
                                   tag="rope_raw", bufs=3)
                    nc.scalar.copy(raw[:, :], ps[:, :])
                    pperm = pps.tile([P, TQ], F32, name="permps",
                                     tag="permps", bufs=1)
                    nc.tensor.matmul(pperm[:, :], lhsT=perm_sb[:, :],
                                     rhs=raw[:, :], start=True, stop=True)
                    t1 = pwk.tile([P, TQ], BF, name="rope_t1",
                                  tag="rope_t1", bufs=3)
                    nc.vector.tensor_mul(t1[:, :], raw[:, :],
                                         cos_sb[:, c * TQ:(c + 1) * TQ])
                    t2 = pwk.tile([P, TQ], BF, name="rope_t2",
                                  tag="rope_t2", bufs=3)
                    nc.vector.tensor_mul(t2[:, :], pperm[:, :],
                                         ssin_sb[:, c * TQ:(c + 1) * TQ])
                    nc.vector.tensor_add(dst[:, c * TQ:(c + 1) * TQ],
                                         t1[:, :], t2[:, :])

            # K (RoPE'd) and V projections first
            for g in range(NKV_LOC):
                rope_head(wkt, g * HD, k_sb[g])
            for t in range(NTT):
                ps = pps.tile([P, NKV_LOC * HD], F32, name="vps",
                              tag="projps", bufs=2)
                for d in range(NDT):
                    nc.tensor.matmul(
                        ps[:, :],
                        lhsT=xt[d][:, t * P:(t + 1) * P],
                        rhs=wvt[d][:, :],
                        start=(d == 0), stop=(d == NDT - 1))
                nc.scalar.copy(v_sb[t][:, :], ps[:, :])

            wo = [None] * NDT
            wo_frees = []

            def attention(h):
                g = h // (NH_LOC // NKV_LOC)
                for qt in range(NQT):
                    nk = (qt + 1) * (TQ // P)
                    sums = pps.tile([1, TQ], F32, name="sums",
                                    tag="sums", bufs=1)
                    opv = pps.tile([P, TQ], F32, name="opv",
                                   tag="opv", bufs=2)
                    for kt in range(nk):
                        sT = pps.tile([P, TQ], F32, name="sT",
                                      tag="sT", bufs=2)
                        nc.tensor.matmul(
                            sT[:, :],
                            lhsT=k_sb[g][:, kt * P:(kt + 1) * P],
                            rhs=q_sb[h][:, qt * TQ:(qt + 1) * TQ],
                            start=True, stop=True)
                        pT = pwk.tile([P, TQ], BF, name="pT",
                                      tag="pT", bufs=6)
                        nc.scalar.activation(
                            pT[:, :], sT[:, :],
                            mybir.ActivationFunctionType.Exp,
                            scale=SCALE)
                        if kt >= nk - (TQ // P):
                            o0 = 384 - P * (kt - (nk - (TQ // P)))
                            nc.vector.tensor_mul(
                                pT[:, :], pT[:, :],
                                mask_sb[:, o0:o0 + TQ])
                        nc.tensor.matmul(
                            sums[:, :], lhsT=ones_sb[:, :], rhs=pT[:, :],
                            start=(kt == 0), stop=(kt == nk - 1))
                        nc.tensor.matmul(
                            opv[:, :],
                            lhsT=v_sb[kt][:, g * HD:(g + 1) * HD],
                            rhs=pT[:, :],
                            start=(kt == 0), stop=(kt == nk - 1))
                    rec = pwk.tile([1, TQ], F32, name="rec",
                                   tag="rec", bufs=2)
                    nc.vector.reciprocal_approx_fast(rec[:, :], sums[:, :])
                    recb = pwk.tile([P, TQ], F32, name="recb",
                                    tag="recb", bufs=2)
                    nc.gpsimd.partition_broadcast(recb[:, :], rec[0:1, :])
                    att = pwk.tile([P, TQ], BF, name="att",
                                   tag="att", bufs=4)
                    nc.vector.tensor_mul(att[:, :], opv[:, :], recb[:, :])
                    c = h // HPC
                    nc.sync.dma_start(
                        out=ag_in[c][(h % HPC) * HD:(h % HPC + 1) * HD,
                                     qt * TQ:(qt + 1) * TQ],
                        in_=att[:, :])

            for h in range(NH_LOC):
                rope_head(wqt, h * HD, q_sb[h])
                if h == NH_LOC - 1:
                    # x^T / Wq / Wkv fully consumed -> free 14MB, load Wo
                    for f_ in reversed(frees):
                        f_()
                    for i in range(NDT):
                        wo[i], wf_ = tc.tile([P, DIM], BF, name=f"wo{i}")
                        wo_frees.append(wf_)
                        nc.sync.dma_start(out=wo[i][:, :],
                                          in_=woT_d[i * P:(i + 1) * P, :])
                attention(h)
                if h % HPC == HPC - 1:
                    c = h // HPC
                    nc.gpsimd.collective_compute(
                        "AllGather",
                        mybir.AluOpType.bypass,
                        replica_groups=[[2 * i, 2 * i + 1] for i in range(4)],
                        ins=[ag_in[c].opt()],
                        outs=[ag_out[c].opt()],
                    )

            # ===== output projection (my token half) =====
            with tc.tile_pool(name="at_pool", bufs=1) as atp, \
                 tc.tile_pool(name="op_wk", bufs=4) as owk:
                at = [atp.tile([P, THALF], BF, name=f"at{f}")
                      for f in range(NDT)]
                for f in range(NDT):
                    r, fl = f // (NDT // 2), f % (NDT // 2)
                    c, row = fl // HPC, fl % HPC
                    nc.sync.dma_start(
                        out=at[f][:, :],
                        in_=ag_out[c][r * HPC * HD + row * P:
                                      r * HPC * HD + (row + 1) * P,
                                      bass.ds(tok0, THALF)])
                for tt in range(8):
                    t0 = tt * P
                    for eh in range(2):
                        ps2 = [pps.tile([P, TQ], F32, name="opps",
                                        tag="sT" if j == 0 else "opv",
                                        bufs=2)
                               for j in range(2)]
                        for f in range(NDT):
                            for j in range(2):
                                e4 = eh * 2 + j
                                nc.tensor.matmul(
                                    ps2[j][:, :],
                                    lhsT=at[f][:, t0:t0 + P],
                                    rhs=wo[f][:, e4 * TQ:(e4 + 1) * TQ],
                                    start=(f == 0), stop=(f == NDT - 1))
                        for j in range(2):
                            e4 = eh * 2 + j
                            osb = owk.tile([P, TQ], mybir.dt.float32,
                                           name="osb", tag="osb", bufs=4)
                            nc.scalar.copy(osb[:, :], ps2[j][:, :])
                            nc.sync.dma_start(
                                out=out_d[t0:t0 + P, e4 * TQ:(e4 + 1) * TQ],
                                in_=osb[:, :])
            for wf_ in reversed(wo_frees):
                wf_()

    nc.compile()
    _PROGRAM_CACHE["nc"] = nc
    return nc


def _host_tables():
    inv_freq = 1.0 / (ROPE_BASE ** (np.arange(0, HD, 2, dtype=np.float64) / HD))
    pos = np.arange(S, dtype=np.float64)
    ang = pos[None, :] * inv_freq[:, None]          # [64, S]
    cos = np.concatenate([np.cos(ang), np.cos(ang)], axis=0)   # [128, S]
    sin = np.sin(ang)
    ssin = np.concatenate([-sin, sin], axis=0)                  # [128, S]

    kk = np.arange(P)[:, None]
    cc = np.arange(1280)[None, :]
    mask = (kk <= cc - 384).astype(np.float32)                  # [128, 1280]

    permm = np.roll(np.eye(P, dtype=np.float32), 64, axis=0)    # q[(d+64)%128]
    ones = np.ones((P, 1), np.float32)
    return (cos.astype(BF16), ssin.astype(BF16), mask.astype(BF16),
            permm.astype(BF16), ones.astype(BF16))


def kernel(x, Wq, Wkv, Wo):
    x = np.asarray(x, np.float32)
    Wq = np.asarray(Wq, np.float32)
    Wkv = np.asarray(Wkv, np.float32)
    Wo = np.asarray(Wo, np.float32)

    nc = _build_program()
    cos, ssin, mask, permm, ones = _host_tables()
    wqT = np.ascontiguousarray(Wq.T).astype(BF16)       # [DIM, 2048]
    wkvT = np.ascontiguousarray(Wkv.T).astype(BF16)     # [DIM, 1024]
    woT = np.ascontiguousarray(Wo.T).astype(BF16)       # [DIM, DIM]

    in_maps = []
    for c in range(N_CORES):
        b, hh = c // 2, c % 2
        xT = np.ascontiguousarray(x[b].T).astype(BF16)  # [DIM, S]
        in_maps.append({
            "xT": xT,
            "wqT": np.ascontiguousarray(
                wqT[:, hh * NH_LOC * HD:(hh + 1) * NH_LOC * HD]),
            "wkT": np.ascontiguousarray(
                wkvT[:, hh * NKV_LOC * HD:(hh + 1) * NKV_LOC * HD]),
            "wvT": np.ascontiguousarray(
                wkvT[:, NKV * HD + hh * NKV_LOC * HD:
                     NKV * HD + (hh + 1) * NKV_LOC * HD]),
            "woT": woT,
            "cos": cos, "ssin": ssin, "mask": mask,
            "perm": permm, "ones": ones,
            "tokoff": np.array([[hh * THALF]], np.uint32),
        })

    trace_kwargs = {}
    if os.environ.get("KERNEL_TRACE") == "1":
        trace_kwargs = dict(trace=True,
                            trace_cores=list(range(N_CORES)),
                            stitch_traces=True)
    elif os.environ.get("KERNEL_TRACE") == "0cores":
        trace_kwargs = dict(trace=True)
    res = run_bass_kernel_spmd(nc, in_maps, core_ids=list(range(N_CORES)),
                               **trace_kwargs)
    _PROGRAM_CACHE["last_results"] = res

    out = np.empty((B, S, DIM), np.float32)
    for c in range(N_CORES):
        b, hh = c // 2, c % 2
        out[b, hh * THALF:(hh + 1) * THALF, :] = res.results[c]["out"]
    return out


# revision 23
# speedup vs baseline: 1.1281x; 1.0233x over previous
"""Causal GQA self-attention (b=4, s=2048, dim=2048, 16 q-heads / 4 kv-heads,
hd=128, RoPE) on 8 TRN2 NeuronCores.

Sharding: tensor-parallel x2 on heads x data-parallel x4 on batch.
Core c <-> (batch c//2, head-half c%2). Each core:
  - projects Q (8 heads) / K,V (2 kv heads) for all 2048 tokens of its batch
    directly in [d, t] layout (host feeds x^T and W^T pre-cast to bf16),
  - applies RoPE via a partition-permutation matmul + DVE combines,
  - computes attention in S^T = K^T.T@Q layout (k on partitions, q on free)
    so softmax needs no transposes: exp on ScalarE, row-sums via a
    ones-column matmul accumulated in PSUM, PV accumulated in PSUM.
    Q-projection of head h+1 is interleaved with attention of head h so
    projection matmuls fill PE stall slots in the attention pipeline,
  - exchanges attention-output head-halves with its pair via 4 chunked
    AllGathers overlapped with attention,
  - runs the output projection for its token half (1024 tokens), full dim;
    the token offset comes from a per-core uint32 input via a dynamic DMA
    offset so the program is identical across cores (SPMD).
Host reassembles the 8 (1024, 2048) f32 slabs.
"""

import os

import numpy as np
import ml_dtypes

import concourse.bass as bass
import concourse.bacc as bacc
import concourse.mybir as mybir
import concourse.tile as tile
from concourse.bass_utils import run_bass_kernel_spmd

BF16 = ml_dtypes.bfloat16
F32 = mybir.dt.float32
BF = mybir.dt.bfloat16

# Problem constants
B, S, DIM = 4, 2048, 2048
NH, NKV, HD = 16, 4, 128
ROPE_BASE = 10000.0
N_CORES = 8

# Per-core layout
NH_LOC = NH // 2          # 8 q heads per core
NKV_LOC = NKV // 2        # 2 kv heads per core
P = 128
NDT = DIM // P            # 16 contraction tiles
TQ = 512                  # q-chunk (free dim of attention matmuls)
NQT = S // TQ             # 4 q-chunks
NTT = S // P              # 16 token tiles of 128
THALF = S // 2            # 1024 tokens per core's output slab
SCALE = 1.0 / float(np.sqrt(HD))
N_CHUNK = 4               # AllGather chunks
HPC = NH_LOC // N_CHUNK   # heads per chunk

_PROGRAM_CACHE = {}


def _build_program():
    if "nc" in _PROGRAM_CACHE:
        return _PROGRAM_CACHE["nc"]

    nc = bacc.Bacc("TRN2", target_bir_lowering=False, debug=False,
                   num_devices=N_CORES)

    xT_d = nc.dram_tensor("xT", [DIM, S], BF, kind="ExternalInput")
    wqT_d = nc.dram_tensor("wqT", [DIM, NH_LOC * HD], BF, kind="ExternalInput")
    wkT_d = nc.dram_tensor("wkT", [DIM, NKV_LOC * HD], BF, kind="ExternalInput")
    wvT_d = nc.dram_tensor("wvT", [DIM, NKV_LOC * HD], BF, kind="ExternalInput")
    woT_d = nc.dram_tensor("woT", [DIM, DIM], BF, kind="ExternalInput")
    cos_d = nc.dram_tensor("cos", [P, S], BF, kind="ExternalInput")
    ssin_d = nc.dram_tensor("ssin", [P, S], BF, kind="ExternalInput")
    mask_d = nc.dram_tensor("mask", [P, 1280], BF, kind="ExternalInput")
    perm_d = nc.dram_tensor("perm", [P, P], BF, kind="ExternalInput")
    ones_d = nc.dram_tensor("ones", [P, 1], BF, kind="ExternalInput")
    tokoff_d = nc.dram_tensor("tokoff", [1, 1], mybir.dt.uint32,
                              kind="ExternalInput")
    out_d = nc.dram_tensor("out", [THALF, DIM], mybir.dt.float32,
                           kind="ExternalOutput")

    with tile.TileContext(nc) as tc:
        with tc.tile_pool(name="const", bufs=1) as constp, \
             tc.tile_pool(name="acts", bufs=1) as actp, \
             tc.tile_pool(name="dram", bufs=1, space="DRAM") as dramp, \
             tc.tile_pool(name="main_ps", bufs=2, space="PSUM") as pps, \
             tc.tile_pool(name="wk", bufs=3) as pwk:
            # ---- constants ----
            cos_sb = constp.tile([P, S], BF, name="cos_sb")
            ssin_sb = constp.tile([P, S], BF, name="ssin_sb")
            mask_sb = constp.tile([P, 1280], BF, name="mask_sb")
            perm_sb = constp.tile([P, P], BF, name="perm_sb")
            ones_sb = constp.tile([P, 1], BF, name="ones_sb")
            tokoff_sb = constp.tile([1, 1], mybir.dt.uint32, name="tokoff_sb")
            nc.sync.dma_start(out=cos_sb[:, :], in_=cos_d[:, :])
            nc.sync.dma_start(out=ssin_sb[:, :], in_=ssin_d[:, :])
            nc.sync.dma_start(out=mask_sb[:, :], in_=mask_d[:, :])
            nc.sync.dma_start(out=perm_sb[:, :], in_=perm_d[:, :])
            nc.sync.dma_start(out=ones_sb[:, :], in_=ones_d[:, :])
            nc.sync.dma_start(out=tokoff_sb[:, :], in_=tokoff_d[:, :])

            treg = nc.sync.alloc_register("tokoff_reg")
            nc.sync.reg_load(treg, tokoff_sb[0:1, 0:1])
            tok0 = nc.sync.snap(treg, donate=True, min_val=0, max_val=THALF)

            # ---- persistent activations ----
            q_sb = [actp.tile([P, S], BF, name=f"q{h}") for h in range(NH_LOC)]
            k_sb = [actp.tile([P, S], BF, name=f"k{g}") for g in range(NKV_LOC)]
            v_sb = [actp.tile([P, NKV_LOC * HD], BF, name=f"v{t}")
                    for t in range(NTT)]

            ag_in = [dramp.tile([HPC * HD, S], BF, name=f"ag_in{c}")
                     for c in range(N_CHUNK)]
            ag_out = [dramp.tile([2 * HPC * HD, S], BF, name=f"ag_out{c}")
                      for c in range(N_CHUNK)]

            # ---- streamed inputs as stack-ordered singles ----
            frees = []
            xt, wqt, wkt, wvt = [], [], [], []
            for i in range(NDT):
                t_, f_ = tc.tile([P, S], BF, name=f"xt{i}")
                xt.append(t_); frees.append(f_)
            for i in range(NDT):
                t_, f_ = tc.tile([P, NH_LOC * HD], BF, name=f"wq{i}")
                wqt.append(t_); frees.append(f_)
            for i in range(NDT):
                t_, f_ = tc.tile([P, NKV_LOC * HD], BF, name=f"wk{i}")
                wkt.append(t_); frees.append(f_)
                t_, f_ = tc.tile([P, NKV_LOC * HD], BF, name=f"wv{i}")
                wvt.append(t_); frees.append(f_)
            for i in range(NDT):
                nc.sync.dma_start(out=xt[i][:, :], in_=xT_d[i * P:(i + 1) * P, :])
                nc.sync.dma_start(out=wkt[i][:, :], in_=wkT_d[i * P:(i + 1) * P, :])
            for i in range(NDT):
                nc.sync.dma_start(out=wvt[i][:, :], in_=wvT_d[i * P:(i + 1) * P, :])
            for i in range(NDT):
                nc.sync.dma_start(out=wqt[i][:, :], in_=wqT_d[i * P:(i + 1) * P, :])

            # PSUM tags: projps(2) + permps(1) + sT(2) + sums(1) + opv(2) = 8
            def rope_head(w_tiles, col0, dst):
                for c in range(NQT):
                    ps = pps.tile([P, TQ], F32, name="projps",
                                  tag="projps", bufs=2)
                    for d in range(NDT):
                        nc.tensor.matmul(
                            ps[:, :],
                            lhsT=w_tiles[d][:, col0:col0 + HD],
                            rhs=xt[d][:, c * TQ:(c + 1) * TQ],
                            start=(d == 0), stop=(d == NDT - 1))
                    raw = pwk.tile([P, TQ], BF, name="rope_raw",
                                   tag="rope_raw", bufs=3)
                    nc.scalar.copy(raw[:, :], ps[:, :])
                    pperm = pps.tile([P, TQ], F32, name="permps",
                                     tag="permps", bufs=1)
                    nc.tensor.matmul(pperm[:, :], lhsT=perm_sb[:, :],
                                     rhs=raw[:, :], start=True, stop=True)
                    t1 = pwk.tile([P, TQ], BF, name="rope_t1",
                                  tag="rope_t1", bufs=3)
                    nc.vector.tensor_mul(t1[:, :], raw[:, :],
                                         cos_sb[:, c * TQ:(c + 1) * TQ])
                    t2 = pwk.tile([P, TQ], BF, name="rope_t2",
                                  tag="rope_t2", bufs=3)
                    nc.vector.tensor_mul(t2[:, :], pperm[:, :],
                                         ssin_sb[:, c * TQ:(c + 1) * TQ])
                    nc.vector.tensor_add(dst[:, c * TQ:(c + 1) * TQ],
                                         t1[:, :], t2[:, :])

            # K (RoPE'd) and V projections first
            for g in range(NKV_LOC):
                rope_head(wkt, g * HD, k_sb[g])
            for t in range(NTT):
                ps = pps.tile([P, NKV_LOC * HD], F32, name="vps",
                              tag="projps", bufs=2)
                for d in range(NDT):
                    nc.tensor.matmul(
                        ps[:, :],
                        lhsT=xt[d][:, t * P:(t + 1) * P],
                        rhs=wvt[d][:, :],
                        start=(d == 0), stop=(d == NDT - 1))
                nc.scalar.copy(v_sb[t][:, :], ps[:, :])

            wo = [None] * NDT
            wo_frees = []

            def attention(h):
                g = h // (NH_LOC // NKV_LOC)
                for qt in range(NQT):
                    nk = (qt + 1) * (TQ // P)
                    sums = pps.tile([1, TQ], F32, name="sums",
                                    tag="sums", bufs=1)
                    opv = pps.tile([P, TQ], F32, name="opv",
                                   tag="opv", bufs=2)
                    for kt in range(nk):
                        sT = pps.tile([P, TQ], F32, name="sT",
                                      tag="sT", bufs=2)
                        nc.tensor.matmul(
                            sT[:, :],
                            lhsT=k_sb[g][:, kt * P:(kt + 1) * P],
                            rhs=q_sb[h][:, qt * TQ:(qt + 1) * TQ],
                            start=True, stop=True)
                        pT = pwk.tile([P, TQ], BF, name="pT",
                                      tag="pT", bufs=6)
                        nc.scalar.activation(
                            pT[:, :], sT[:, :],
                            mybir.ActivationFunctionType.Exp,
                            scale=SCALE)
                        if kt >= nk - (TQ // P):
                            o0 = 384 - P * (kt - (nk - (TQ // P)))
                            nc.vector.tensor_mul(
                                pT[:, :], pT[:, :],
                                mask_sb[:, o0:o0 + TQ])
                        nc.tensor.matmul(
                            sums[:, :], lhsT=ones_sb[:, :], rhs=pT[:, :],
                            start=(kt == 0), stop=(kt == nk - 1))
                        nc.tensor.matmul(
                            opv[:, :],
                            lhsT=v_sb[kt][:, g * HD:(g + 1) * HD],
                            rhs=pT[:, :],
                            start=(kt == 0), stop=(kt == nk - 1))
                    rec = pwk.tile([1, TQ], F32, name="rec",
                                   tag="rec", bufs=2)
                    nc.vector.reciprocal_approx_fast(rec[:, :], sums[:, :])
                    recb = pwk.tile([P, TQ], F32, name="recb",
                                    tag="recb", bufs=2)
                    nc.gpsimd.partition_broadcast(recb[:, :], rec[0:1, :])
                    att = pwk.tile([P, TQ], BF, name="att",
                                   tag="att", bufs=4)
                    nc.vector.tensor_mul(att[:, :], opv[:, :], recb[:, :])
                    c = h // HPC
                    nc.sync.dma_start(
                        out=ag_in[c][(h % HPC) * HD:(h % HPC + 1) * HD,
                                     qt * TQ:(qt + 1) * TQ],
                        in_=att[:, :])

            for h in range(NH_LOC):
                rope_head(wqt, h * HD, q_sb[h])
                if h == NH_LOC - 1:
                    # x^T / Wq / Wkv fully consumed -> free 14MB, load Wo
                    for f_ in reversed(frees):
                        f_()
                    for i in range(NDT):
                        wo[i], wf_ = tc.tile([P, DIM], BF, name=f"wo{i}")
                        wo_frees.append(wf_)
                        nc.sync.dma_start(out=wo[i][:, :],
                                          in_=woT_d[i * P:(i + 1) * P, :])
                attention(h)
                if h % HPC == HPC - 1:
                    c = h // HPC
                    nc.gpsimd.collective_compute(
                        "AllGather",
                        mybir.AluOpType.bypass,
                        replica_groups=[[2 * i, 2 * i + 1] for i in range(4)],
                        ins=[ag_in[c].opt()],
                        outs=[ag_out[c].opt()],
                    )

            # ===== output projection (my token half) =====
            with tc.tile_pool(name="at_pool", bufs=1) as atp, \
                 tc.tile_pool(name="op_wk", bufs=4) as owk:
                at = [atp.tile([P, THALF], BF, name=f"at{f}")
                      for f in range(NDT)]
                for f in range(NDT):
                    r, fl = f // (NDT // 2), f % (NDT // 2)
                    c, row = fl // HPC, fl % HPC
                    nc.sync.dma_start(
                        out=at[f][:, :],
                        in_=ag_out[c][r * HPC * HD + row * P:
                                      r * HPC * HD + (row + 1) * P,
                                      bass.ds(tok0, THALF)])
                ford = [fl + r * (NDT // 2)
                        for c in range(N_CHUNK)
                        for r in range(2)
                        for fl in range(c * HPC, (c + 1) * HPC)]
                for tt in range(8):
                    t0 = tt * P
                    for eh in range(2):
                        ps2 = [pps.tile([P, TQ], F32, name="opps",
                                        tag="projps" if j == 0 else "permps",
                                        bufs=2 if j == 0 else 1)
                               for j in range(2)]
                        for fi, f in enumerate(ford):
                            for j in range(2):
                                e4 = eh * 2 + j
                                nc.tensor.matmul(
                                    ps2[j][:, :],
                                    lhsT=at[f][:, t0:t0 + P],
                                    rhs=wo[f][:, e4 * TQ:(e4 + 1) * TQ],
                                    start=(fi == 0), stop=(fi == NDT - 1))
                        for j in range(2):
                            e4 = eh * 2 + j
                            osb = owk.tile([P, TQ], mybir.dt.float32,
                                           name="osb", tag="osb", bufs=4)
                            nc.scalar.copy(osb[:, :], ps2[j][:, :])
                            nc.sync.dma_start(
                                out=out_d[t0:t0 + P, e4 * TQ:(e4 + 1) * TQ],
                                in_=osb[:, :])
            for wf_ in reversed(wo_frees):
                wf_()

    nc.compile()
    _PROGRAM_CACHE["nc"] = nc
    return nc


def _host_tables():
    inv_freq = 1.0 / (ROPE_BASE ** (np.arange(0, HD, 2, dtype=np.float64) / HD))
    pos = np.arange(S, dtype=np.float64)
    ang = pos[None, :] * inv_freq[:, None]          # [64, S]
    cos = np.concatenate([np.cos(ang), np.cos(ang)], axis=0)   # [128, S]
    sin = np.sin(ang)
    ssin = np.concatenate([-sin, sin], axis=0)                  # [128, S]

    kk = np.arange(P)[:, None]
    cc = np.arange(1280)[None, :]
    mask = (kk <= cc - 384).astype(np.float32)                  # [128, 1280]

    permm = np.roll(np.eye(P, dtype=np.float32), 64, axis=0)    # q[(d+64)%128]
    ones = np.ones((P, 1), np.float32)
    return (cos.astype(BF16), ssin.astype(BF16), mask.astype(BF16),
            permm.astype(BF16), ones.astype(BF16))


def kernel(x, Wq, Wkv, Wo):
    x = np.asarray(x, np.float32)
    Wq = np.asarray(Wq, np.float32)
    Wkv = np.asarray(Wkv, np.float32)
    Wo = np.asarray(Wo, np.float32)

    nc = _build_program()
    cos, ssin, mask, permm, ones = _host_tables()
    wqT = np.ascontiguousarray(Wq.T).astype(BF16)       # [DIM, 2048]
    wkvT = np.ascontiguousarray(Wkv.T).astype(BF16)     # [DIM, 1024]
    woT = np.ascontiguousarray(Wo.T).astype(BF16)       # [DIM, DIM]

    in_maps = []
    for c in range(N_CORES):
        b, hh = c // 2, c % 2
        xT = np.ascontiguousarray(x[b].T).astype(BF16)  # [DIM, S]
        in_maps.append({
            "xT": xT,
            "wqT": np.ascontiguousarray(
                wqT[:, hh * NH_LOC * HD:(hh + 1) * NH_LOC * HD]),
            "wkT": np.ascontiguousarray(
                wkvT[:, hh * NKV_LOC * HD:(hh + 1) * NKV_LOC * HD]),
            "wvT": np.ascontiguousarray(
                wkvT[:, NKV * HD + hh * NKV_LOC * HD:
                     NKV * HD + (hh + 1) * NKV_LOC * HD]),
            "woT": woT,
            "cos": cos, "ssin": ssin, "mask": mask,
            "perm": permm, "ones": ones,
            "tokoff": np.array([[hh * THALF]], np.uint32),
        })

    trace_kwargs = {}
    if os.environ.get("KERNEL_TRACE") == "1":
        trace_kwargs = dict(trace=True,
                            trace_cores=list(range(N_CORES)),
                            stitch_traces=True)
    elif os.environ.get("KERNEL_TRACE") == "0cores":
        trace_kwargs = dict(trace=True)
    res = run_bass_kernel_spmd(nc, in_maps, core_ids=list(range(N_CORES)),
                               **trace_kwargs)
    _PROGRAM_CACHE["last_results"] = res

    out = np.empty((B, S, DIM), np.float32)
    for c in range(N_CORES):
        b, hh = c // 2, c % 2
        out[b, hh * THALF:(hh + 1) * THALF, :] = res.results[c]["out"]
    return out
